# revision 1
# baseline (speedup 1.0000x reference)
"""Trainium2 Bass kernel for the dendritic-branch spiking FNN (DH_SFNN).

Model (per reference):
  branch_in = x @ W_in.T + b_in                  # (B,T,H*BR)
  per t:  i_d = beta*i_d + (1-beta)*branch_in_t  # beta = sigmoid(tau_n), (H,BR)
          v   = alpha*v + (1-alpha)*i_d.sum(br)  # alpha = sigmoid(tau_m), (H,)
          spike = (v >= 1); v -= spike; counts += spike
  out = counts @ W_out.T + b_out                 # (B,D_OUT)

Strategy: data-parallel over batch across 8 cores (32 rows each). Per core,
T=500 is processed in chunks (9x50 + 2x25; the small tail chunks shrink the
serial epilogue after the last matmul), pipelined across engines:
  PE : fp16-split 3-pass GEMM (full fp32-quality; spikes are rare so the
       threshold dynamics need ~1e-5 relative accuracy on branch_in --
       fp16x1/fp32r/fp16x2 all flip spikes and fail the 2e-2 gate)
  Act: PSUM->SBUF epilogue applying (1-beta)(1-alpha) scale + bias, and the
       Sign() spike-compare on the history buffer
  Pool(gpsimd): branch-sum adds (the only heavy op walrus allows on Pool)
  DVE: IIR scans, carry handling, the serial per-timestep spike/reset loop,
       and the spike-count time reduction
u/wti/hist are double-buffered so chunk c+1's GEMM overlaps chunk c's
scan/spike work. Spike counts are recovered from sum_t sign(v'_t - 1) with
the affine correction folded into the readout weights/bias on the host.
"""

import sys

if "/opt/trn_rl_repo" not in sys.path:
    sys.path.insert(0, "/opt/trn_rl_repo")

from contextlib import ExitStack

import numpy as np

import concourse.bass as bass
import concourse.mybir as mybir
import concourse.tile as tile
from concourse import bacc

B, T, D_IN, H, BR, D_OUT = 256, 500, 700, 200, 2, 35
NCORES = 8
BL = B // NCORES          # local batch = 32
NK = 6                    # k-tiles; D_IN padded 700 -> 768 so every tile is 128
DP = NK * 128             # padded contraction dim (768)
M = 4                     # m-tiles, m=(br,j): o'' = m*128 + p, h = (m%2)*128+p
OP = M * 128              # padded output rows (512)
NJ = 2                    # h groups (j=0: h<128, j=1: h 128..199)
NF = NJ * BL              # spike-loop state columns (64)
BG = 8                    # batches per matmul n-group
NG = BL // BG             # 4 n-groups

CHUNKS = (50,) * 9 + (25, 15, 10)     # sum = T; shrinking tail chunks


def _f32(a):
    return np.ascontiguousarray(a, dtype=np.float32)


def _build(T_, chunks, alpha_uniform_val=None, reps=1, add_eng="pool",
           red_eng="dve", cmp_eng="act"):
    """Build the single-core Bass program.
    add_eng:  engine for the branch-sum adds ("pool"|"dve"); Pool/gpsimd
              only supports tensor_tensor/tensor_copy/tensor_scalar (walrus
              rejects scalar_tensor_tensor and tensor_tensor_scan on Pool)
    red_eng:  engine for the spike-count time reduction ("dve" only for
              free-axis reduces)
    cmp_eng:  engine for the spike compare over hist ("act"|"dve")
    """
    chunks = tuple(chunks)
    NCH = len(chunks)
    assert sum(chunks) == T_
    C0 = max(chunks)
    sizes = sorted(set(chunks))
    # x columns per (chunk, group); chunk col offsets in the flat x tensor
    xoff = np.cumsum([0] + [NK * BG * cc for cc in chunks]).tolist()
    FT = xoff[-1]
    # d0 blocks per distinct chunk size
    soff = {}
    off = 0
    for s in sizes:
        soff[s] = off
        off += M * BL * s
    SD = off

    fp32 = mybir.dt.float32
    fp16 = mybir.dt.float16
    AF = mybir.ActivationFunctionType
    AL = mybir.AluOpType

    nc = bacc.Bacc("TRN2", target_bir_lowering=False, debug=False,
                   num_devices=NCORES)

    nxs = 2                                 # x/w operand copies (hi, lo)
    xt_d = nc.dram_tensor("xt", [nxs, NG, 128, FT], fp16, kind="ExternalInput")
    wt_d = nc.dram_tensor("wt", [nxs, NK, 128, OP], fp16, kind="ExternalInput")
    sc2_d = nc.dram_tensor("sc2", [128, M], fp32, kind="ExternalInput")
    b2_d = nc.dram_tensor("b2", [128, M], fp32, kind="ExternalInput")
    bt_d = nc.dram_tensor("bt", [128, M], fp32, kind="ExternalInput")
    atile_d = nc.dram_tensor("atile", [128, NF], fp32, kind="ExternalInput")
    woutT_d = nc.dram_tensor("woutT", [2 * 128, D_OUT], fp32, kind="ExternalInput")
    bout_d = nc.dram_tensor("bout", [D_OUT, 1], fp32, kind="ExternalInput")

    out_d = nc.dram_tensor("out", [D_OUT, BL], fp32, kind="ExternalOutput")
    # tiny passthrough tensor so benchmark harnesses can chain executions
    tok_d = nc.dram_tensor("tok", [1, 16], fp32, kind="ExternalInput")
    tok_o = nc.dram_tensor("tok_out", [1, 16], fp32, kind="ExternalOutput")

    with tile.TileContext(nc) as tc, ExitStack() as ctx:
        const = ctx.enter_context(tc.tile_pool(name="const", bufs=1))
        st = ctx.enter_context(tc.tile_pool(name="state", bufs=1))
        up = ctx.enter_context(tc.tile_pool(name="up", bufs=2))
        wp = ctx.enter_context(tc.tile_pool(name="wph", bufs=2))
        hp = ctx.enter_context(tc.tile_pool(name="hp", bufs=2))
        xp = ctx.enter_context(tc.tile_pool(name="xin", bufs=3))
        ps = ctx.enter_context(tc.tile_pool(name="psum", bufs=6, space="PSUM"))
        pso = ctx.enter_context(tc.tile_pool(name="psout", bufs=1, space="PSUM"))
        scr = ctx.enter_context(tc.tile_pool(name="scr", bufs=2))

        w_sbs = []
        for s in range(nxs):
            w_sb_s = const.tile([128, NK * OP], fp16, tag=f"wsb{s}")
            # w-lo is first needed 12 matmuls in; issue it on the Act queue
            # so the SP queue goes straight from w-hi to the first x tiles
            eng = nc.sync if s == 0 else nc.scalar
            eng.dma_start(
                w_sb_s[:].rearrange("p (k o) -> p k o", k=NK),
                wt_d.ap()[s].rearrange("k p o -> p k o"))
            w_sbs.append(w_sb_s)
        sc2 = const.tile([128, M], fp32)
        nc.sync.dma_start(sc2[:], sc2_d.ap())
        b2 = const.tile([128, M], fp32)
        nc.sync.dma_start(b2[:], b2_d.ap())
        # big/late-needed consts go on the Act HWDGE queue so the SP queue
        # reaches the first x tiles immediately
        bt = const.tile([128, M], fp32)
        nc.scalar.dma_start(bt[:], bt_d.ap())
        atile = const.tile([128, NF], fp32)
        nc.scalar.dma_start(atile[:], atile_d.ap())
        # d0 (scan multipliers: beta everywhere, 0 at each batch's t=0) is
        # generated on device: Act broadcasts bt per partition, DVE zeroes
        # the chunk-start columns -- saves a 38KB/partition DMA at startup
        d0_sb = const.tile([128, SD], fp32)
        # memset first: the Act broadcast reads blk with scale=0, and
        # 0 * garbage(Inf/NaN) = NaN if SBUF holds prior-program junk
        nc.vector.memset(d0_sb[:], 0.0)
        for si, s in enumerate(sizes):
            for m in range(M):
                blk = d0_sb[:, soff[s] + m * BL * s:
                            soff[s] + (m + 1) * BL * s]
                nc.scalar.activation(blk, blk, AF.Identity,
                                     bias=bt[:, m:m + 1], scale=0.0)
                nc.vector.memset(
                    blk.rearrange("p (b c) -> p b c", c=s)[:, :, 0], 0.0)
        woutT_sb = const.tile([128, 2 * D_OUT], fp32)
        nc.scalar.dma_start(woutT_sb[:, 0:D_OUT], woutT_d.ap()[0:128])
        nc.scalar.dma_start(woutT_sb[:, D_OUT:2 * D_OUT], woutT_d.ap()[128:256])
        bout_sb = const.tile([D_OUT, 1], fp32)
        nc.scalar.dma_start(bout_sb[:], bout_d.ap())
        negone = const.tile([128, 1], fp32, tag="negone")
        nc.vector.memset(negone[:], -1.0)

        e_add = nc.gpsimd if add_eng == "pool" else nc.vector
        e_red = nc.gpsimd if red_eng == "pool" else nc.vector

        def cmp_phase(hist, cc):
            # spike count via sum_t sign(v'-1): hist holds -v' (pre-reset)
            hv = hist[:, 0:cc * NF]
            nc.scalar.activation(hv, hv, AF.Sign,
                                 bias=negone[:, 0:1], scale=negone[:, 0:1])

        def red_phase(hist, cc, counts):
            csc = scr.tile([128, NF], fp32, tag="csc")
            e_red.tensor_reduce(
                csc[:], hist[:, 0:cc * NF].rearrange("p (c f) -> p f c", f=NF),
                mybir.AxisListType.X, AL.add)
            e_red.tensor_tensor(counts[:], counts[:], csc[:], AL.add)

        def body_once():
            vst = st.tile([128, NF], fp32, tag="vst")  # negated potential
            counts = st.tile([128, NF], fp32, tag="cnt")
            carry = st.tile([128, M * BL], fp32, tag="carry")
            nc.vector.memset(vst[:], 0.0)
            nc.vector.memset(counts[:], 0.0)

            prev = None     # (hist, chunk_len) of previous chunk
            for c, CC in enumerate(chunks):
                NNc = BG * CC
                u = up.tile([128, M * BL * C0], fp32, tag="u")
                wti = wp.tile([128, C0 * NF], fp32, tag="wti")
                hist = hp.tile([128, C0 * NF], fp32, tag="hist")

                # -- GEMM: u[m-tile, b, t] = scaled(x @ W') --
                for g in range(NG):
                    x_sbs = []
                    for s in range(nxs):
                        x_sb_s = xp.tile([128, NK * BG * C0], fp16,
                                         tag=f"xsb{s}")
                        nc.sync.dma_start(
                            x_sb_s[:, 0:NK * NNc],
                            xt_d.ap()[s, g][:, xoff[c]:xoff[c + 1]])
                        x_sbs.append(x_sb_s)
                    pairs = [(0, 0), (0, 1), (1, 0)]      # hh, hl, lh
                    for m in range(M):
                        pt = ps.tile([128, NNc], fp32, tag="pt")
                        nmm = len(pairs) * NK
                        i = 0
                        for (ws, xs) in pairs:
                            for k in range(NK):
                                nc.tensor.matmul(
                                    pt[:],
                                    w_sbs[ws][:, k * OP + m * 128:
                                              k * OP + (m + 1) * 128],
                                    x_sbs[xs][:, k * NNc:(k + 1) * NNc],
                                    start=(i == 0), stop=(i == nmm - 1))
                                i += 1
                        # u = (1-beta)(1-alpha) * (x@W + b_in), i.e. the IIR
                        # input prescaled so w = i0+i1 needs no extra scaling
                        nc.scalar.activation(
                            u[:, m * BL * C0 + g * NNc:
                              m * BL * C0 + (g + 1) * NNc],
                            pt[:], AF.Identity,
                            bias=b2[:, m:m + 1], scale=sc2[:, m:m + 1])

                # spike-compare of previous chunk (emitted here so Act's
                # Sign op sits between this chunk's and next chunk's psum
                # epilogues, never blocking the GEMM stream)
                if prev is not None:
                    cmp_phase(*prev)

                # -- dendrite IIR: i_d = beta*i_d + u, fused scan per m-tile --
                for m in range(M):
                    um = u[:, m * BL * C0:m * BL * C0 + BL * CC]
                    um3 = um.rearrange("p (b c) -> p b c", c=CC)
                    if c > 0:
                        # u[:, b, 0] += beta * carry_b
                        nc.vector.scalar_tensor_tensor(
                            um3[:, :, 0], carry[:, m * BL:(m + 1) * BL],
                            bt[:, m:m + 1], um3[:, :, 0], AL.mult, AL.add)
                    d0c = soff[CC] + m * BL * CC
                    nc.vector.tensor_tensor_scan(
                        um[:], d0_sb[:, d0c:d0c + BL * CC],
                        um[:], 0.0, AL.mult, AL.add)
                    if c < NCH - 1:
                        nc.vector.tensor_copy(carry[:, m * BL:(m + 1) * BL],
                                              um3[:, :, CC - 1])

                # count-reduce of previous chunk: placed after the scans so
                # DVE never waits on Act's Sign (which runs concurrently)
                if prev is not None:
                    red_phase(prev[0], prev[1], counts)

                # -- branch sum: w_j = i'_d[j] + i'_d[2+j] (pre-scaled) --
                # wti stored t-major: col = t*NF + j*BL + b
                wre = wti[:, 0:CC * NF].rearrange("p (c j b) -> p b j c",
                                                  j=NJ, b=BL)
                for j in range(NJ):
                    e_add.tensor_tensor(
                        wre[:, :, j, :],
                        u[:, j * BL * C0:j * BL * C0 + BL * CC].rearrange(
                            "p (b c) -> p b c", c=CC),
                        u[:, (2 + j) * BL * C0:(2 + j) * BL * C0 + BL * CC
                          ].rearrange("p (b c) -> p b c", c=CC), AL.add)

                # -- spike loop (negated state: vt = -v) --
                for t in range(CC):
                    tA = hist[:, t * NF:(t + 1) * NF]   # pre-reset vt' kept
                    wt_t = wti[:, t * NF:(t + 1) * NF]
                    if alpha_uniform_val is not None:
                        # vt' = alpha*vt - w_t
                        nc.vector.scalar_tensor_tensor(
                            tA, vst[:], float(alpha_uniform_val),
                            wt_t, AL.mult, AL.subtract)
                    else:
                        nc.vector.tensor_tensor(tA, vst[:], atile[:], AL.mult)
                        nc.vector.tensor_tensor(tA, tA, wt_t, AL.subtract)
                    # vt'' = (vt' <= -1) + vt'   (spike subtract, negated)
                    nc.vector.scalar_tensor_tensor(
                        vst[:], tA, -1.0, tA, AL.is_le, AL.add)
                prev = (hist, CC)

            # final chunk: compare inline on DVE -- skips the Act sem
            # round-trip on the critical serial tail
            fh, fcc = prev
            nc.vector.tensor_scalar(fh[:, 0:fcc * NF], fh[:, 0:fcc * NF],
                                    -1.0, None, AL.is_le)
            csc = scr.tile([128, NF], fp32, tag="csc")
            nc.vector.tensor_reduce(
                csc[:], fh[:, 0:fcc * NF].rearrange("p (c f) -> p f c", f=NF),
                mybir.AxisListType.X, AL.add)
            # is_le gives {0,1}: counts_true = sum directly; readout expects
            # sign-sum S = 2*counts - T, so S = 2*red - T
            nc.vector.scalar_tensor_tensor(
                counts[:], csc[:], 2.0, counts[:], AL.mult, AL.add)

            # -- readout: out = woutT'^T @ S + bout' (count transform folded
            #    on host: counts = 0.5*S + 0.5*T) --
            po = pso.tile([D_OUT, BL], fp32, tag="po")
            nc.tensor.matmul(po[:], woutT_sb[:, 0:D_OUT], counts[:, 0:BL],
                             start=True, stop=False)
            nc.tensor.matmul(po[:], woutT_sb[0:H - 128, D_OUT:2 * D_OUT],
                             counts[0:H - 128, BL:2 * BL], start=False,
                             stop=True)
            out_sb = scr.tile([D_OUT, BL], fp32, tag="osb")
            nc.scalar.activation(out_sb[:], po[:], AF.Identity,
                                 bias=bout_sb[:, 0:1], scale=1.0)
            nc.sync.dma_start(out_d.ap(), out_sb[:])

        if reps == 1:
            body_once()
        else:
            with tc.For_i(0, reps, 1):
                body_once()
        tok_sb = scr.tile([1, 16], fp32, tag="tok")
        nc.sync.dma_start(tok_sb[:], tok_d.ap())
        nc.sync.dma_start(tok_o.ap(), tok_sb[:])

    nc.compile()
    return nc


def _prep_host(x, W_in, b_in, tau_n, tau_m, W_out, b_out, T_, chunks):
    """Host-side constant prep. Returns (shared_inputs, per_core_x, alpha_uni)."""
    x = _f32(x); W_in = _f32(W_in); b_in = _f32(b_in)
    tau_n = _f32(tau_n); tau_m = _f32(tau_m)
    W_out = _f32(W_out); b_out = _f32(b_out)
    chunks = tuple(chunks)
    assert sum(chunks) == T_

    beta = _f32(1.0 / (1.0 + np.exp(-tau_n.astype(np.float64))))   # (H,BR)
    alpha = _f32(1.0 / (1.0 + np.exp(-tau_m.astype(np.float64))))  # (H,)
    one = np.float32(1.0)

    # m-tile map: m=(br,j) -> rows p: h = (m%2)*128+p, o = h*BR + br
    wt = np.zeros((NK, 128, OP), np.float32)
    sc2 = np.zeros((128, M), np.float32)
    b2 = np.zeros((128, M), np.float32)
    bt = np.zeros((128, M), np.float32)
    for m in range(M):
        br, j = m // 2, m % 2
        for p in range(128):
            h = j * 128 + p
            if h >= H:
                continue
            o = h * BR + br
            s = (one - beta[h, br]) * (one - alpha[h])
            sc2[p, m] = s
            b2[p, m] = s * b_in[o]
            bt[p, m] = beta[h, br]
            wrow = np.zeros(DP, np.float32)
            wrow[:D_IN] = W_in[o]
            wt[:, :, m * 128 + p] = wrow.reshape(NK, 128)
    atile = np.zeros((128, NF), np.float32)
    for j in range(NJ):
        for p in range(128):
            h = j * 128 + p
            if h >= H:
                continue
            atile[p, j * BL:(j + 1) * BL] = alpha[h]
    # readout with the sign-sum transform folded in: chunks except the last
    # contribute sign-sums (2c - len), the last contributes 2c directly, so
    # counts = 0.5*S + 0.5*(T - last)
    woutT = np.zeros((256, D_OUT), np.float32)
    woutT[:H, :] = 0.5 * W_out.T
    teff = np.float32(T_ - chunks[-1])
    bout = (b_out + 0.5 * teff * W_out.sum(axis=1)).reshape(D_OUT, 1)

    def _split16(a):
        hi = a.astype(np.float16)
        lo = (a - hi.astype(np.float32)).astype(np.float16)
        return np.stack([hi, lo])

    shared = dict(wt=_split16(wt), sc2=sc2, b2=b2, bt=bt, atile=atile,
                  woutT=_f32(woutT), bout=_f32(bout))
    FT = sum(NK * BG * cc for cc in chunks)
    xts = []
    for core in range(NCORES):
        xl = x[core * BL:(core + 1) * BL, :T_, :]        # (BL,T,D_IN)
        xp_ = np.zeros((BL, T_, DP), np.float32)
        xp_[:, :, :D_IN] = xl
        xt = np.zeros((NG, 128, FT), np.float32)
        colo = 0
        t0 = 0
        for cc in chunks:
            sub = xp_[:, t0:t0 + cc, :]                  # (BL, cc, DP)
            for g in range(NG):
                sg = sub[g * BG:(g + 1) * BG]            # (BG, cc, DP)
                # -> (128p, NK, BG, cc) -> flat (k, bi, t)
                sg = sg.reshape(BG, cc, NK, 128).transpose(3, 2, 0, 1)
                xt[g, :, colo:colo + NK * BG * cc] = sg.reshape(128, -1)
            colo += NK * BG * cc
            t0 += cc
        xts.append(_split16(xt))
    uni = float(alpha[0]) if np.all(alpha == alpha[0]) else None
    return shared, xts, uni


TRACE = False          # set by test harness for profiling runs
LAST_RESULT = None


def kernel(x, W_in, b_in, tau_n, tau_m, W_out, b_out):
    global LAST_RESULT
    from concourse.bass_utils import run_bass_kernel_spmd

    shared, xts, uni = _prep_host(x, W_in, b_in, tau_n, tau_m, W_out, b_out,
                                  T, CHUNKS)
    nc = _build(T, CHUNKS, alpha_uniform_val=uni)
    tok = np.zeros((1, 16), np.float32)
    in_maps = [dict(shared, xt=xts[core], tok=tok) for core in range(NCORES)]
    res = run_bass_kernel_spmd(nc, in_maps, core_ids=list(range(NCORES)),
                               trace=TRACE)
    LAST_RESULT = res
    out = np.empty((B, D_OUT), np.float32)
    for core in range(NCORES):
        out[core * BL:(core + 1) * BL, :] = res.results[core]["out"].T
    return out

